# revision 27
# baseline (speedup 1.0000x reference)
"""Bahdanau attention (with coverage) Trainium2 Bass kernel.

Problem (full shapes): B=32, L=2048, E=1024, U=1024
  pre   = enc_output @ Ws + Ws_b + (dec_hidden @ Wh + Wh_b)[:,None,:]
          (+ prev_coverage @ Wc + Wc_b  if use_coverage)
  score = tanh(pre) @ V + V_b                      # [B, L, 1]
  attn  = softmax(score * mask, axis=1)
  cov   = attn + prev_coverage (if use_coverage else attn)
  ctx   = sum_l attn * enc_output                  # [B, E]

Sharding: data-parallel over batch: 8 cores x 4 batches each. Each core is
fully independent (no collectives); host splits inputs / concatenates
outputs.

Per-core dataflow (b in 0..3, l-chunks of 512):
  - X^T tiles built on-chip with PE transposes (fp32, via identity matmul)
  - pre^T [U_part, L_free] accumulated in PSUM with fp32r matmuls
    (stationary = Ws block, moving = X^T chunk); coverage term added as a
    K=1 rank-1 matmul (Wc row (x) coverage row); per-U bias
    (h@Wh + Wh_b + Ws_b [+Wc_b]) folded into the tanh activation's
    per-partition bias operand.
  - scores via PE matvec (stationary = V block, moving = tanh tiles)
  - row softmax on [1, 2048] (DVE reduce + ACT exp with fused sum)
  - context via PE: stationary = attn^T column, moving = X natural tiles
  All per-batch row vectors (cov/mask/scores/attn/ctx) are kept in a
  partition-0 row layout [1, B_LOC, *] so K=1 matmul operands sit at base
  partition 0 (engines cannot move data across partitions).
"""

import numpy as np

P = 128
B_LOC = 4  # batches per core
L = 2048
E = 1024
U = 1024
LC = 512  # l-chunk size
N_LC = L // LC  # 4
EO = E // P  # 8
UO = U // P  # 8
N_CORES = 8


def build_nc(use_coverage: bool, reps: int = 1, loop_n: int | None = None):
    import contextlib

    import concourse.bass as bass
    import concourse.mybir as mybir
    import concourse.tile as tile
    from concourse import bacc
    from concourse.masks import make_identity

    f32 = mybir.dt.float32
    f32r = mybir.dt.float32r
    bf16 = mybir.dt.bfloat16
    AF = mybir.ActivationFunctionType

    nc = bacc.Bacc("TRN2", target_bir_lowering=False, debug=False)

    # ---- DRAM I/O (per-core shard) ----
    x_d = nc.dram_tensor("x", [B_LOC, L, E], f32, kind="ExternalInput")
    h_d = nc.dram_tensor("h", [B_LOC, E], f32, kind="ExternalInput")
    cov_d = nc.dram_tensor("cov", [B_LOC, L], f32, kind="ExternalInput")
    mask_d = nc.dram_tensor("mask", [B_LOC, L], f32, kind="ExternalInput")
    ws_d = nc.dram_tensor("ws", [E, U], f32, kind="ExternalInput")
    wh_d = nc.dram_tensor("wh", [E, U], f32, kind="ExternalInput")
    wc_d = nc.dram_tensor("wc", [1, U], f32, kind="ExternalInput")
    v_d = nc.dram_tensor("v", [U], f32, kind="ExternalInput")
    wsb_d = nc.dram_tensor("ws_b", [1, U], f32, kind="ExternalInput")
    whb_d = nc.dram_tensor("wh_b", [1, U], f32, kind="ExternalInput")
    wcb_d = nc.dram_tensor("wc_b", [1, U], f32, kind="ExternalInput")
    vb_d = nc.dram_tensor("v_b", [1, 1], f32, kind="ExternalInput")

    ctx_d = nc.dram_tensor("ctx_out", [B_LOC, E], f32, kind="ExternalOutput")
    attn_d = nc.dram_tensor("attn_out", [B_LOC, L], f32, kind="ExternalOutput")
    covout_d = nc.dram_tensor("cov_out", [B_LOC, L], f32, kind="ExternalOutput")

    with tile.TileContext(nc) as tc:
        with (
            tc.tile_pool(name="const", bufs=1) as constp,
            tc.tile_pool(name="xnat", bufs=2) as xnatp,
            tc.tile_pool(name="xt", bufs=3) as xtp,
            tc.tile_pool(name="tanh", bufs=2) as tanhp,
            tc.tile_pool(name="xctx", bufs=8) as xctxp,
            tc.tile_pool(name="small", bufs=2) as smallp,
            tc.tile_pool(name="rows", bufs=2) as rowsp,
            tc.tile_pool(name="rows1", bufs=1) as rows1p,
            tc.tile_pool(name="dram", bufs=1, space="DRAM") as dramp,
            tc.tile_pool(name="ps_t", bufs=2, space="PSUM") as ps_tp,
            tc.tile_pool(name="ps_pre", bufs=3, space="PSUM") as ps_prep,
            tc.tile_pool(name="ps_vec", bufs=3, space="PSUM") as ps_vecp,
        ):
            loop_cm = tc.For_i(0, loop_n, 1) if loop_n else contextlib.nullcontext()
            with loop_cm:
              for _rep in range(reps):
                # ---- constants / setup ----
                xbf = dramp.tile([B_LOC, L, E], bf16)
                prefetched = set()
                for _lc in range(2):
                    nc.gpsimd.dma_start(
                        xbf[0, _lc * LC : (_lc + 1) * LC, :],
                        x_d[0, _lc * LC : (_lc + 1) * LC, :],
                    )
                    prefetched.add((0, _lc))

                ws_sb = constp.tile([P, EO, U], bf16)  # [ei, eo, u]
                for eo in range(EO):
                    stg = xnatp.tile([P, E], f32, tag="xn")
                    nc.sync.dma_start(
                        stg[:],
                        ws_d[:].rearrange("(eo ei) u -> ei eo u", ei=P)[:, eo, :],
                    )
                    nc.vector.tensor_copy(ws_sb[:, eo, :], stg[:])
                v_sb = constp.tile([P, UO], f32)  # [ui, uo]
                nc.sync.dma_start(v_sb[:], v_d[:].rearrange("(uo ui) -> ui uo", ui=P))
                v_bf = constp.tile([P, UO], bf16)
                nc.vector.tensor_copy(v_bf[:], v_sb[:])
                wc_stg = smallp.tile([1, U], f32, tag="brow")
                nc.sync.dma_start(wc_stg[:], wc_d[:])
                wc_sb = constp.tile([1, U], bf16)
                nc.vector.tensor_copy(wc_sb[:], wc_stg[:])
                vb_sb = constp.tile([1, 1], f32)
                nc.sync.dma_start(vb_sb[:], vb_d[:])
                h_sb = constp.tile([B_LOC, E], f32)
                nc.sync.dma_start(h_sb[:], h_d[:])

                identity = constp.tile([P, P], f32)
                make_identity(nc, identity[:])
                ones_b = constp.tile([1, B_LOC], f32)
                nc.vector.memset(ones_b[:], 1.0)
                ones_1 = constp.tile([1, 1], f32)
                nc.vector.memset(ones_1[:], 1.0)

                # bias_sum[u] = Ws_b + Wh_b (+ Wc_b)
                bias_sum = constp.tile([1, U], f32)
                b1 = smallp.tile([1, U], f32, tag="brow")
                nc.sync.dma_start(b1[:], wsb_d[:])
                b2 = smallp.tile([1, U], f32, tag="brow")
                nc.sync.dma_start(b2[:], whb_d[:])
                nc.vector.tensor_add(bias_sum[:], b1[:], b2[:])
                if use_coverage:
                    b3 = smallp.tile([1, U], f32, tag="brow")
                    nc.sync.dma_start(b3[:], wcb_d[:])
                    nc.vector.tensor_add(bias_sum[:], bias_sum[:], b3[:])

                # h^T: [ei, eo, b]
                hT = constp.tile([P, EO, B_LOC], f32)
                for eh in range(2):
                    pst = ps_tp.tile([P, 4 * P], f32, tag="tp")
                    for eq in range(4):
                        eo = eh * 4 + eq
                        nc.tensor.transpose(
                            pst[:, eq * P : eq * P + B_LOC],
                            h_sb[:, eo * P : (eo + 1) * P],
                            identity[:B_LOC, :B_LOC],
                        )
                        nc.vector.tensor_copy(
                            hT[:, eo, :], pst[:, eq * P : eq * P + B_LOC]
                        )

                # row_biasT[u_part, uo, b] = (h @ Wh)[b, u] + bias_sum[u]
                row_biasT = constp.tile([P, UO, B_LOC], f32)
                for uo in range(UO):
                    wh_blk = xnatp.tile([P, EO, P], f32, tag="xn")
                    nc.sync.dma_start(
                        wh_blk[:],
                        wh_d[:].rearrange("(eo ei) u -> ei eo u", ei=P)[
                            :, :, uo * P : (uo + 1) * P
                        ],
                    )
                    pr = ps_prep.tile([P, LC], f32, tag="pre")
                    for eo in range(EO):
                        nc.tensor.matmul(
                            pr[:, :B_LOC],
                            wh_blk[:, eo, :],
                            hT[:, eo, :],
                            start=(eo == 0),
                            stop=False,
                        )
                    nc.tensor.matmul(
                        pr[:, :B_LOC],
                        bias_sum[0:1, uo * P : (uo + 1) * P],
                        ones_b[:],
                        start=False,
                        stop=True,
                    )
                    nc.vector.tensor_copy(row_biasT[:, uo, :], pr[:, :B_LOC])

                # per-batch softmax scalars (partition 0)
                rmax = constp.tile([1, B_LOC], f32)
                negmax = constp.tile([1, B_LOC], f32)
                sumexp = constp.tile([1, B_LOC], f32)
                rsum = constp.tile([1, B_LOC], f32)

                def emit_phase1(b):
                    st = {"b": b}
                    covb = rowsp.tile([1, L], f32, tag="covr")
                    nc.sync.dma_start(covb[:], cov_d[None, b, :])
                    covb_r = rows1p.tile([1, L], bf16, tag="covrr")
                    nc.gpsimd.dma_start(covb_r[:], cov_d[None, b, :])
                    maskb = rows1p.tile([1, L], f32, tag="maskr")
                    nc.sync.dma_start(maskb[:], mask_d[None, b, :])
                    scoresb = rowsp.tile([1, L], f32, tag="scoresr")
                    st.update(covb=covb, maskb=maskb, scoresb=scoresb)

                    def emit_xts(lc):
                        l0 = lc * LC
                        xts = xtp.tile([P, EO, LC], bf16, tag="xt")
                        if (b, lc) not in prefetched:
                            nc.gpsimd.dma_start(
                                xbf[b, l0 : l0 + LC, :], x_d[b, l0 : l0 + LC, :]
                            )
                        for eo in range(EO):
                            nc.sync.dma_start(
                                xts[:, eo, :],
                                xbf[b, l0 : l0 + LC, eo * P : (eo + 1) * P],
                                transpose=True,
                            )
                        return xts

                    def emit_chunk_mms(lc, xts):
                        l0 = lc * LC
                        th = tanhp.tile([P, UO, LC], bf16, tag="th")
                        for up in range(UO // 2):
                            uos = (2 * up, 2 * up + 1)
                            pp0 = ps_prep.tile([P, LC], f32, tag="pre", name="pp0")
                            pp1 = ps_prep.tile([P, LC], f32, tag="pre", name="pp1")
                            pps = [pp0, pp1]
                            for eo in range(EO):
                                for pp, uo in zip(pps, uos):
                                    nc.tensor.matmul(
                                        pp[:],
                                        ws_sb[:, eo, uo * P : (uo + 1) * P],
                                        xts[:, eo, :],
                                        start=(eo == 0),
                                        stop=(eo == EO - 1 and not use_coverage),
                                    )
                            for pp, uo in zip(pps, uos):
                                if use_coverage:
                                    nc.tensor.matmul(
                                        pp[:],
                                        wc_sb[0:1, uo * P : (uo + 1) * P],
                                        covb_r[0:1, l0 : l0 + LC],
                                        start=False,
                                        stop=True,
                                    )
                                nc.scalar.activation(
                                    th[:, uo, :],
                                    pp[:],
                                    AF.Tanh,
                                    bias=row_biasT[:, uo, b : b + 1],
                                    scale=1.0,
                                )
                        ps_s = ps_vecp.tile([1, LC], f32, tag="vec")
                        for uo in range(UO):
                            nc.tensor.matmul(
                                ps_s[:],
                                v_bf[:, uo : uo + 1],
                                th[:, uo, :],
                                start=(uo == 0),
                                stop=(uo == UO - 1),
                            )
                        nc.vector.tensor_scalar_add(
                            scoresb[0:1, l0 : l0 + LC], ps_s[:], vb_sb[0:1, 0:1]
                        )

                    prev_x = None
                    for lc in range(N_LC):
                        xts = emit_xts(lc)
                        if prev_x is not None:
                            emit_chunk_mms(prev_x[0], prev_x[1])
                        prev_x = (lc, xts)
                    emit_chunk_mms(prev_x[0], prev_x[1])
                    return st

                def emit_phase23(st):
                    b = st["b"]
                    covb, maskb, scoresb = st["covb"], st["maskb"], st["scoresb"]
                    attnb = rowsp.tile([1, L], f32, tag="attnr")
                    # softmax over l (row [1, 2048])
                    nc.vector.tensor_tensor(
                        scoresb[0:1, :],
                        scoresb[0:1, :],
                        maskb[0:1, :],
                        mybir.AluOpType.mult,
                    )
                    nc.vector.reduce_max(
                        rmax[0:1, b : b + 1],
                        scoresb[0:1, :],
                        axis=mybir.AxisListType.X,
                    )
                    nc.vector.tensor_scalar_mul(
                        negmax[0:1, b : b + 1], rmax[0:1, b : b + 1], -1.0
                    )
                    nc.scalar.activation(
                        attnb[0:1, :],
                        scoresb[0:1, :],
                        AF.Exp,
                        bias=negmax[0:1, b : b + 1],
                        scale=1.0,
                        accum_out=sumexp[0:1, b : b + 1],
                    )
                    nc.vector.reciprocal(rsum[0:1, b : b + 1], sumexp[0:1, b : b + 1])
                    nc.vector.tensor_scalar_mul(
                        attnb[0:1, :], attnb[0:1, :], rsum[0:1, b : b + 1]
                    )
                    nc.sync.dma_start(attn_d[None, b, :], attnb[0:1, :])
                    covoutb = rows1p.tile([1, L], f32, tag="covoutr")
                    if use_coverage:
                        nc.vector.tensor_add(
                            covoutb[0:1, :], attnb[0:1, :], covb[0:1, :]
                        )
                    else:
                        nc.vector.tensor_copy(covoutb[0:1, :], attnb[0:1, :])
                    nc.sync.dma_start(covout_d[None, b, :], covoutb[0:1, :])

                    # attn^T columns + context
                    at = smallp.tile([P, L // P], bf16, tag="at")
                    for lq in range(L // P // 4):
                        pst = ps_tp.tile([P, 4 * P], f32, tag="tp")
                        for i in range(4):
                            lt = lq * 4 + i
                            nc.tensor.matmul(
                                pst[:, i : i + 1],
                                attnb[0:1, lt * P : (lt + 1) * P],
                                ones_1[:],
                                start=True,
                                stop=True,
                            )
                        nc.vector.tensor_copy(
                            at[:, lq * 4 : (lq + 1) * 4], pst[:, 0:4]
                        )

                    ctxb = rowsp.tile([1, E], f32, tag="ctxr")
                    pc0 = ps_vecp.tile([1, LC], f32, tag="vec")
                    pc1 = ps_vecp.tile([1, LC], f32, tag="vec")
                    for lt in range(L // P):
                        xc = xctxp.tile([P, E], bf16, tag="xc")
                        nc.sync.dma_start(xc[:], xbf[b, lt * P : (lt + 1) * P, :])
                        nc.tensor.matmul(
                            pc0[:],
                            at[:, lt : lt + 1],
                            xc[:, 0:LC],
                            start=(lt == 0),
                            stop=(lt == L // P - 1),
                        )
                        nc.tensor.matmul(
                            pc1[:],
                            at[:, lt : lt + 1],
                            xc[:, LC:E],
                            start=(lt == 0),
                            stop=(lt == L // P - 1),
                        )
                    nc.vector.tensor_copy(ctxb[0:1, 0:LC], pc0[:])
                    nc.vector.tensor_copy(ctxb[0:1, LC:E], pc1[:])
                    nc.sync.dma_start(ctx_d[None, b, :], ctxb[0:1, :])

                prev = None
                for b in range(B_LOC):
                    st = emit_phase1(b)
                    if prev is not None:
                        emit_phase23(prev)
                    prev = st
                emit_phase23(prev)

    nc.compile()
    return nc


def make_in_maps(inputs):
    """Split full inputs into 8 per-core input maps."""
    x = np.ascontiguousarray(np.asarray(inputs["enc_output"], dtype=np.float32))
    h = np.ascontiguousarray(np.asarray(inputs["dec_hidden"], dtype=np.float32))
    cov = np.ascontiguousarray(
        np.asarray(inputs["prev_coverage"], dtype=np.float32)[:, :, 0]
    )
    mask = np.ascontiguousarray(np.asarray(inputs["enc_pad_mask"]).astype(np.float32))
    ws = np.ascontiguousarray(np.asarray(inputs["Ws_k"], dtype=np.float32))
    wh = np.ascontiguousarray(np.asarray(inputs["Wh_k"], dtype=np.float32))
    wc = np.ascontiguousarray(np.asarray(inputs["Wc_k"], dtype=np.float32))
    v = np.ascontiguousarray(np.asarray(inputs["V_k"], dtype=np.float32)[:, 0])
    wsb = np.asarray(inputs["Ws_b"], dtype=np.float32).reshape(1, U)
    whb = np.asarray(inputs["Wh_b"], dtype=np.float32).reshape(1, U)
    wcb = np.asarray(inputs["Wc_b"], dtype=np.float32).reshape(1, U)
    vb = np.asarray(inputs["V_b"], dtype=np.float32).reshape(1, 1)

    in_maps = []
    for c in range(N_CORES):
        s = slice(c * B_LOC, (c + 1) * B_LOC)
        in_maps.append(
            {
                "x": x[s],
                "h": h[s],
                "cov": cov[s],
                "mask": mask[s],
                "ws": ws,
                "wh": wh,
                "wc": wc,
                "v": v,
                "ws_b": wsb,
                "wh_b": whb,
                "wc_b": wcb,
                "v_b": vb,
            }
        )
    return in_maps


def run_on_hw(inputs, trace=False):
    from concourse.bass_utils import run_bass_kernel_spmd

    use_coverage = bool(int(np.asarray(inputs["use_coverage"])))
    nc = build_nc(use_coverage)
    in_maps = make_in_maps(inputs)
    res = run_bass_kernel_spmd(nc, in_maps, list(range(N_CORES)), trace=trace)

    ctx = np.concatenate([r["ctx_out"] for r in res.results], axis=0)
    attn = np.concatenate([r["attn_out"] for r in res.results], axis=0)[..., None]
    covo = np.concatenate([r["cov_out"] for r in res.results], axis=0)[..., None]
    return (ctx, attn, covo), res


def kernel(**inputs):
    outs, _ = run_on_hw(inputs, trace=False)
    return outs
